# revision 1
# baseline (speedup 1.0000x reference)
"""Trainium2 Bass kernel for nn_MultiHeadAttention_73589969649754
(gnn_message_passing / graph cross-attention).

v3 strategy (score sharing + matmul scoring):
  - Edges sorted by destination per side; core c owns nodes
    [c*2500, (c+1)*2500) on both sides.  Windows of <=128 consecutive
    nodes / <=1024 edge slots.  The HOST permutes each core's node
    columns so window w occupies slots [w*128, (w+1)*128) - all program
    offsets are SPMD-uniform and tables are slot-ordered.
  - Phase A: R side emits a fused K|V table (AllGather #1, 5.24MB/rank);
    L side computes K TRANSPOSED ([channel, slot], kept in SBUF - no
    DRAM round trip) and a V table (AllGather #2, 2.62MB/rank).
  - L pass per window: one TRANSPOSED dma_gather pulls Kr[src] in
    [channel, edge] orientation and a normal gather pulls Vr[src].
    Scores come from PE matmuls M[slot, edge] = KlT_win^T @ KrT_gath
    (no per-edge dot product on DVE/ACT), then exp on ACT, a host-shipped
    one-hot mask zeroes off-segment entries, and a PE transpose yields
    E^T[edge, slot] - which IS the eh-scaled one-hot the segment-sum
    matmuls consume.  eh per edge (row-sum of E^T) is written to a score
    shard (AllGather #3, 80KB/rank).  z via ones-matmul; messages are
    normalized after the fact by 1/z (column broadcast); Wo GEMM;
    bias+LeakyReLU.
  - R pass: gathers only Vl[src] rows (1KB/edge) plus 256B score-table
    rows; a host-shipped one-hot mask selects each edge's eh (softmax
    numerators are shared between the two sides).  No K gather, no dot
    product, no exp on the R side.
  - Outputs stored bf16 in [channel, slot] layout; host reassembles.
"""

import math

import numpy as np

N = 20000
E = 160000
C = 512
NCORES = 8
TEMP = float(np.sqrt(C))
NEG = 0.01
NPC = N // NCORES            # 2500 nodes per core per side
BLK = 128                    # edges per block
BPW = 8                      # blocks per window
WCAP = BPW * BLK             # 1024 edge slots per window
DUMMY_REL = 999.0
SKIP_AG = False
SKIP_C = False
SKIP_L = False       # debug: skip the L window loop
SKIP_R = False       # debug: skip the R window loop

# AllGather wall-time charge (ns): measured-table upper bound is ~70us for
# a 5.24MB/rank 8-way intra-chip AllGather; scale by bytes with a 20us
# latency floor.  Serial charges: AG#1 tkv_R 5.24MB -> 70us (gates the L
# loop) and AG#3 escore 80KB -> 20us (floor; gates the R score gathers).
# AG#2 (tv_L, 2.62MB -> ~35us) runs on the collective cores/links right
# after AG#1 (~105us done) and is only consumed by the R loop, which
# starts after the ~175us L loop - fully hidden, so not charged.
AG_CHARGE_NS = 70e3 + 20e3


def _prep_side(seg_dst):
    """Sort edges by dst; node-aligned core ranges; pack windows."""
    seg_dst = np.asarray(seg_dst, np.int64)
    perm = np.argsort(seg_dst, kind="stable")
    sd = seg_dst[perm]
    deg = np.bincount(sd, minlength=N)
    edge_b = [int(np.searchsorted(sd, c * NPC, "left"))
              for c in range(NCORES)] + [E]

    cores = []
    max_w = 0
    for c in range(NCORES):
        n0, n1 = c * NPC, (c + 1) * NPC
        e0 = edge_b[c]
        wins = []
        n, e = n0, e0
        while n < n1:
            wn = we = 0
            while n + wn < n1 and wn < BLK and we + deg[n + wn] <= WCAP:
                we += deg[n + wn]
                wn += 1
            assert wn > 0, "node degree exceeds window capacity"
            wins.append((n, wn, e, we))
            n += wn
            e += we
        assert e == edge_b[c + 1]
        cores.append((wins, e0))
        max_w = max(max_w, len(wins))
    return perm, sd, cores, max_w


def _wrap_idx16(idx_flat):
    """[n] -> [128, n//16] int16, i at [i%16, i//16], replicated x8."""
    n = idx_flat.shape[0]
    a = idx_flat.reshape(n // 16, 16).T.astype(np.int16)
    return np.ascontiguousarray(np.tile(a, (8, 1)))


def _slot_maps(cores, W):
    """Slot-order the nodes: window w of core c occupies slots
    [w*128, (w+1)*128).  Returns (slot_node [NCORES, W*128] node-or--1,
    node_slot [N] global slot = core*W*128 + slot)."""
    npad = W * BLK
    slot_node = np.full((NCORES, npad), -1, np.int64)
    node_slot = np.full(N, -1, np.int64)
    for c, (wins, e0) in enumerate(cores):
        for w, (fn, wn, es, ne) in enumerate(wins):
            sl = np.arange(wn)
            slot_node[c, w * BLK + sl] = fn + sl
            node_slot[fn + sl] = c * npad + w * BLK + sl
    assert (node_slot >= 0).all()
    return slot_node, node_slot


def _edge_locs(perm, cores):
    """Per original edge id: (core, window, slot-in-window) on this side."""
    core_of = np.empty(E, np.int32)
    win_of = np.empty(E, np.int32)
    slot_of = np.empty(E, np.int32)
    for c, (wins, e0) in enumerate(cores):
        for w, (fn, wn, es, ne) in enumerate(wins):
            ids = perm[es:es + ne]
            core_of[ids] = c
            win_of[ids] = w
            slot_of[ids] = np.arange(ne)
    return core_of, win_of, slot_of


def _host_inputs(inputs):
    import ml_dtypes
    bf16 = ml_dtypes.bfloat16

    nl = np.asarray(inputs["node_left"], np.float32)
    nr = np.asarray(inputs["node_right"], np.float32)
    Wk = np.asarray(inputs["Wk"], np.float32)
    Wv = np.asarray(inputs["Wv"], np.float32)
    Wo = np.asarray(inputs["Wo"], np.float32)
    bo = np.asarray(inputs["bo"], np.float32)
    sl = np.asarray(inputs["segmentation_index_left"], np.int64)
    sr = np.asarray(inputs["segmentation_index_right"], np.int64)

    permL, sdL, coresL, wL = _prep_side(sl)
    permR, sdR, coresR, wR = _prep_side(sr)
    W = max(wL, wR)
    npad = W * BLK

    snL, nsL = _slot_maps(coresL, W)     # left-node slots
    snR, nsR = _slot_maps(coresR, W)     # right-node slots
    LcoreE, LwinE, LslotE = _edge_locs(permL, coresL)

    # ---- per-core L arrays ----
    ssL = sr[permL]          # src (right) node per L-sorted edge
    arrL = []
    for c, (wins, e0) in enumerate(coresL):
        sidx = np.zeros((W, WCAP), np.int64)      # src rows in R table
        drel = np.full((W, WCAP), DUMMY_REL, np.float32)
        qtm = np.zeros((BLK, W * WCAP), np.float32)
        for w, (fn, wn, es, ne) in enumerate(wins):
            sidx[w, :ne] = nsR[ssL[es:es + ne]]
            rel = (sdL[es:es + ne] - fn)
            drel[w, :ne] = rel.astype(np.float32)
            # QT mask: [dst-rel, (w*8+b)*128 + p] = 1 for edge slot i=b*128+p
            i = np.arange(ne)
            qtm[rel, w * WCAP + (i // BLK) * BLK + (i % BLK)] = 1.0
        sidx16 = np.concatenate([_wrap_idx16(sidx[w]) for w in range(W)], 1)
        drelT = np.ascontiguousarray(
            drel.reshape(W, BPW, BLK).transpose(2, 0, 1).reshape(BLK, W * BPW))
        arrL.append(dict(sidx=sidx16, drel=drelT, qtm=qtm.astype(bf16),
                         colnode=snL[c]))

    # ---- per-core R arrays ----
    ssR = sl[permR]          # src (left) node per R-sorted edge
    arrR = []
    for c, (wins, e0) in enumerate(coresR):
        vidx = np.zeros((W, WCAP), np.int64)      # src rows in L V table
        eidx = np.zeros((W, WCAP), np.int64)      # escore row (64-col rows)
        ecol = np.zeros((W, WCAP), np.int64)
        emask_valid = np.zeros((W, WCAP), bool)
        drel = np.full((W, WCAP), DUMMY_REL, np.float32)
        for w, (fn, wn, es, ne) in enumerate(wins):
            ids = permR[es:es + ne]
            vidx[w, :ne] = nsL[ssR[es:es + ne]]
            # flat L escore position: core*(W*1024) + win*1024 + p*8 + b
            pL = LslotE[ids] % BLK
            bL = LslotE[ids] // BLK
            gpos = (LcoreE[ids].astype(np.int64) * W + LwinE[ids]) * WCAP \
                + pL * BPW + bL
            eidx[w, :ne] = gpos // 64
            ecol[w, :ne] = gpos % 64
            emask_valid[w, :ne] = True
            drel[w, :ne] = (sdR[es:es + ne] - fn).astype(np.float32)
        vidx16 = np.concatenate([_wrap_idx16(vidx[w]) for w in range(W)], 1)
        eidx16 = np.concatenate([_wrap_idx16(eidx[w]) for w in range(W)], 1)
        drelT = np.ascontiguousarray(
            drel.reshape(W, BPW, BLK).transpose(2, 0, 1).reshape(BLK, W * BPW))
        emask = np.zeros((BLK, W * BPW * 64), np.float32)
        wv, iv = np.nonzero(emask_valid)
        pv, bv = iv % BLK, iv // BLK
        emask[pv, (wv * BPW + bv) * 64 + ecol[wv, iv]] = 1.0
        arrR.append(dict(vidx=vidx16, eidx=eidx16, drel=drelT,
                         emask=emask.astype(bf16), colnode=snR[c]))

    # ---- shared constants ----
    Wkv = np.concatenate([Wk, Wv], 0)               # [1024, 512]
    WkvT = Wkv.T                                    # [512, 1024]
    wkvT_arr = np.zeros((128, 4 * 1024), np.float32)
    for cc in range(4):
        wkvT_arr[:, cc * 1024:(cc + 1) * 1024] = \
            WkvT[cc * 128:(cc + 1) * 128, :]
    # wkT: lhsT tiles for the transposed K GEMM
    wkT_arr = np.zeros((128, 16 * 128), np.float32)
    for i in range(4):
        for o in range(4):
            wkT_arr[:, (i * 4 + o) * 128:(i * 4 + o + 1) * 128] = \
                Wk[o * 128:(o + 1) * 128, i * 128:(i + 1) * 128].T
    woT_arr = np.zeros((128, 4 * 512), np.float32)
    for cc in range(4):
        for oc in range(4):
            woT_arr[:, cc * 512 + oc * 128: cc * 512 + (oc + 1) * 128] = \
                Wo[oc * 128:(oc + 1) * 128, cc * 128:(cc + 1) * 128].T
    bo_arr = bo.reshape(4, 128).T.copy()            # [128, 4]
    iota_arr = np.broadcast_to(
        np.arange(128, dtype=np.float32)[None, :], (128, 128))
    ident_arr = np.eye(128, dtype=np.float32)

    def shardT(feat, slot_node_c):
        sh = np.zeros((C, npad), np.float32)
        m = slot_node_c >= 0
        sh[:, m] = feat[slot_node_c[m]].T
        return np.ascontiguousarray(sh).astype(bf16)

    in_maps = []
    for c in range(NCORES):
        in_maps.append({
            "nT_L": shardT(nl, snL[c]),
            "nT_R": shardT(nr, snR[c]),
            "wkvT": wkvT_arr.astype(bf16),
            "wkT": wkT_arr.astype(bf16),
            "woT": woT_arr.astype(bf16),
            "bo": bo_arr,
            "iota": np.ascontiguousarray(iota_arr).astype(bf16),
            "ident": np.ascontiguousarray(ident_arr).astype(bf16),
            "sidx_L": arrL[c]["sidx"],
            "drel_L": arrL[c]["drel"],
            "qtm_L": arrL[c]["qtm"],
            "vidx_R": arrR[c]["vidx"],
            "eidx_R": arrR[c]["eidx"],
            "drel_R": arrR[c]["drel"],
            "emask_R": arrR[c]["emask"],
        })
    return in_maps, arrL, arrR, W


def _build_program(W):
    import concourse.bacc as bacc
    import concourse.tile as tile
    from concourse import mybir

    dt = mybir.dt
    f32, bf16, i16 = dt.float32, dt.bfloat16, dt.int16
    AF = mybir.ActivationFunctionType
    OP = mybir.AluOpType
    npad = W * BLK
    ntiles = W

    nc = bacc.Bacc("TRN2", target_bir_lowering=False, debug=False,
                   enable_asserts=True, num_devices=NCORES,
                   num_swdge_queues=2)

    # ---- I/O ----
    nT_in = {s: nc.dram_tensor(f"nT_{s}", [C, npad], bf16,
                               kind="ExternalInput").ap() for s in "LR"}
    wkvT = nc.dram_tensor("wkvT", [128, 4 * 1024], bf16,
                          kind="ExternalInput").ap()
    wkT_in = nc.dram_tensor("wkT", [128, 16 * 128], bf16,
                            kind="ExternalInput").ap()
    woT = nc.dram_tensor("woT", [128, 4 * 512], bf16,
                         kind="ExternalInput").ap()
    bo_in = nc.dram_tensor("bo", [128, 4], f32, kind="ExternalInput").ap()
    iota_in = nc.dram_tensor("iota", [128, 128], bf16,
                             kind="ExternalInput").ap()
    ident_in = nc.dram_tensor("ident", [128, 128], bf16,
                              kind="ExternalInput").ap()
    sidx_in = nc.dram_tensor("sidx_L", [128, W * 64], i16,
                             kind="ExternalInput").ap()
    drelL_in = nc.dram_tensor("drel_L", [128, W * BPW], f32,
                              kind="ExternalInput").ap()
    qtm_in = nc.dram_tensor("qtm_L", [128, W * WCAP], bf16,
                            kind="ExternalInput").ap()
    vidx_in = nc.dram_tensor("vidx_R", [128, W * 64], i16,
                             kind="ExternalInput").ap()
    eidx_in = nc.dram_tensor("eidx_R", [128, W * 64], i16,
                             kind="ExternalInput").ap()
    drelR_in = nc.dram_tensor("drel_R", [128, W * BPW], f32,
                              kind="ExternalInput").ap()
    emask_in = nc.dram_tensor("emask_R", [128, W * BPW * 64], bf16,
                              kind="ExternalInput").ap()
    hT_out = {s: nc.dram_tensor(f"hT_{s}", [C, npad], bf16,
                                kind="ExternalOutput").ap() for s in "LR"}

    # ---- internal DRAM ----
    tkv_shR = nc.dram_tensor("tkv_shR", [npad, 2 * C], bf16).ap()
    tv_shL = nc.dram_tensor("tv_shL", [npad, C], bf16).ap()
    esc_sh = nc.dram_tensor("esc_sh", [W, 128, BPW], f32).ap()
    tkv_R = nc.dram_tensor("tkv_R", [NCORES * npad, 2 * C], bf16,
                           addr_space="Shared").ap()
    tv_L = nc.dram_tensor("tv_L", [NCORES * npad, C], bf16,
                          addr_space="Shared").ap()
    esc_full = nc.dram_tensor("esc_full", [NCORES * W * 16, 64], f32,
                              addr_space="Shared").ap()

    with tile.TileContext(nc) as tc:
        with tc.tile_pool(name="const", bufs=1) as cpool:
            # early constants (phase A + L loop)
            wkvT_sb = cpool.tile([128, 4 * 1024], bf16)
            nc.sync.dma_start(wkvT_sb[:], wkvT[:, :])
            wkT_sb = cpool.tile([128, 16 * 128], bf16)
            nc.sync.dma_start(wkT_sb[:], wkT_in[:, :])
            sidx_sb = cpool.tile([128, W * 64], i16)
            nc.sync.dma_start(sidx_sb[:], sidx_in[:, :])
            drelL_sb = cpool.tile([128, W * BPW], f32)
            nc.sync.dma_start(drelL_sb[:], drelL_in[:, :])
            ident_sb = cpool.tile([128, 128], bf16)
            nc.sync.dma_start(ident_sb[:], ident_in[:, :])
            woT_sb = cpool.tile([128, 4 * 512], bf16)
            nc.sync.dma_start(woT_sb[:], woT[:, :])
            bo_sb = cpool.tile([128, 4], f32)
            nc.sync.dma_start(bo_sb[:], bo_in[:, :])
            iota_sb = cpool.tile([128, 128], bf16)
            nc.sync.dma_start(iota_sb[:], iota_in[:, :])
            ones_col = cpool.tile([128, 1], bf16)
            nc.vector.memset(ones_col[:], 1.0)
            ones_row = cpool.tile([1, 128], bf16)
            nc.vector.memset(ones_row[:], 1.0)
            # late constants (R loop only; loaded after phase A issues)
            vidx_sb = cpool.tile([128, W * 64], i16)
            eidx_sb = cpool.tile([128, W * 64], i16)
            drelR_sb = cpool.tile([128, W * BPW], f32)
            emask_sb = cpool.tile([128, W * BPW * 64], bf16)
            hacc = cpool.tile([128, 4 * npad], bf16)
            klT_sb = cpool.tile([128, 4, npad], bf16)   # left K transposed

            # ---- phase A ----
            with (
                tc.tile_pool(name="feat", bufs=1) as fpool,
                tc.tile_pool(name="gemm_sb", bufs=3) as gsb,
                tc.tile_pool(name="psum_gemm", bufs=2, space="PSUM") as pg,
            ):
                featR = []
                for cc in range(4):
                    t = fpool.tile([128, npad], bf16, tag=f"featR{cc}")
                    nc.sync.dma_start(
                        t[:], nT_in["R"][cc * 128:(cc + 1) * 128, :])
                    featR.append(t)
                # R side K|V fused GEMM -> tkv_shR -> AG#1
                for ti in range(ntiles):
                    sb = gsb.tile([128, 1024], bf16)
                    for half in range(2):
                        ps = pg.tile([128, 512], f32)
                        for cc in range(4):
                            nc.tensor.matmul(
                                ps[:],
                                lhsT=featR[cc][:, ti * 128:(ti + 1) * 128],
                                rhs=wkvT_sb[:, cc * 1024 + half * 512:
                                            cc * 1024 + half * 512 + 512],
                                start=(cc == 0), stop=(cc == 3))
                        if half == 0:
                            nc.vector.tensor_copy(
                                sb[:, 0:512], ps[:])
                        else:
                            nc.scalar.copy(
                                sb[:, 512:1024], ps[:])
                    nc.sync.dma_start(
                        tkv_shR[ti * 128:(ti + 1) * 128, :], sb[:])
                if not SKIP_AG:
                    nc.gpsimd.collective_compute(
                        "AllGather", mybir.AluOpType.bypass,
                        replica_groups=[list(range(NCORES))],
                        ins=[tkv_shR], outs=[tkv_R])
                else:
                    # timing build: sliver copy keeps the dependency edge
                    nc.sync.dma_start(tkv_R[0:128, :], tkv_shR[0:128, :])

                featL = []
                for cc in range(4):
                    t = fpool.tile([128, npad], bf16, tag=f"featL{cc}")
                    nc.sync.dma_start(
                        t[:], nT_in["L"][cc * 128:(cc + 1) * 128, :])
                    featL.append(t)
                # L side transposed-K GEMM -> klT_sb (stays in SBUF)
                for o in range(4):
                    for nb in range(math.ceil(npad / 512)):
                        n0 = nb * 512
                        n1 = min(npad, n0 + 512)
                        ps = pg.tile([128, 512], f32)
                        for i in range(4):
                            nc.tensor.matmul(
                                ps[:, 0:n1 - n0],
                                lhsT=wkT_sb[:, (i * 4 + o) * 128:
                                            (i * 4 + o + 1) * 128],
                                rhs=featL[i][:, n0:n1],
                                start=(i == 0), stop=(i == 3))
                        nc.scalar.copy(
                            klT_sb[:, o, n0:n1], ps[:, 0:n1 - n0])
                # L side V GEMM -> tv_shL -> AG#2
                for ti in range(ntiles):
                    sb = gsb.tile([128, 512], bf16, tag="sbv")
                    ps = pg.tile([128, 512], f32)
                    for cc in range(4):
                        nc.tensor.matmul(
                            ps[:],
                            lhsT=featL[cc][:, ti * 128:(ti + 1) * 128],
                            rhs=wkvT_sb[:, cc * 1024 + 512:
                                        cc * 1024 + 1024],
                            start=(cc == 0), stop=(cc == 3))
                    nc.vector.tensor_copy(sb[:], ps[:])
                    nc.sync.dma_start(
                        tv_shL[ti * 128:(ti + 1) * 128, :], sb[:])
                if not SKIP_AG:
                    nc.gpsimd.collective_compute(
                        "AllGather", mybir.AluOpType.bypass,
                        replica_groups=[list(range(NCORES))],
                        ins=[tv_shL], outs=[tv_L])
                else:
                    nc.sync.dma_start(tv_L[0:128, :], tv_shL[0:128, :])

            # late const loads (R loop)
            nc.sync.dma_start(vidx_sb[:], vidx_in[:, :])
            nc.sync.dma_start(eidx_sb[:], eidx_in[:, :])
            nc.sync.dma_start(drelR_sb[:], drelR_in[:, :])
            nc.sync.dma_start(emask_sb[:], emask_in[:, :])

            nidx_reg = nc.gpsimd.to_reg(WCAP)
            nidx_reg2 = nc.gpsimd.to_reg(WCAP // 2)

            with (
                tc.tile_pool(name="gath", bufs=3) as gpool,
                tc.tile_pool(name="aux2", bufs=3) as kdpool,
                tc.tile_pool(name="qtm", bufs=3) as qpool,
                tc.tile_pool(name="blk", bufs=4) as sp,
                tc.tile_pool(name="ebuf", bufs=4) as ebpool,
                tc.tile_pool(name="etbuf", bufs=2 * BPW) as ohpool,
                tc.tile_pool(name="tail", bufs=3) as tp,
                tc.tile_pool(name="pMT", bufs=2, space="PSUM") as pMT,
                tc.tile_pool(name="pmsg", bufs=2, space="PSUM") as pmsg,
                tc.tile_pool(name="paux", bufs=1, space="PSUM") as paux,
                tc.tile_pool(name="ph", bufs=1, space="PSUM") as ph,
            ):
                # ---- phase C-L: matmul scores + left messages ----
                for w in ([] if (SKIP_C or SKIP_L) else range(W)):
                    # transposed K gather split in two: the worker's ucode
                    # fails above 512 indices per transpose gather
                    ktgs = []
                    for h in range(2):
                        kt = gpool.tile([128, 4, WCAP // 2], bf16,
                                        tag=f"ktg{h}")
                        nc.gpsimd.dma_gather(
                            kt[:], tkv_R[:, 0:C],
                            sidx_sb[:, w * 64 + h * 32: w * 64 + h * 32 + 32],
                            WCAP // 2, nidx_reg2, C, elem_step=2 * C,
                            transpose=True)
                        ktgs.append(kt)
                    vg = gpool.tile([128, BPW, C], bf16, tag="vg")
                    nc.gpsimd.dma_gather(
                        vg[:], tkv_R[:, C:2 * C],
                        sidx_sb[:, w * 64:(w + 1) * 64],
                        WCAP, nidx_reg, C, elem_step=2 * C, queue_num=1)
                    qtm = qpool.tile([128, WCAP], bf16, tag="qtm")
                    nc.sync.dma_start(
                        qtm[:], qtm_in[:, w * WCAP:(w + 1) * WCAP])

                    ehs = sp.tile([128, BPW], f32, tag="ehs")
                    ets = []
                    msgT_ps = pmsg.tile([128, 512], f32)
                    z_ps = paux.tile([128, 128], f32, tag="aux")
                    for b in range(BPW):
                        M_ps = pMT.tile([128, 128], f32, tag="M")
                        kth = ktgs[b // 4]
                        bh = b % 4
                        for j in range(4):
                            nc.tensor.matmul(
                                M_ps[:],
                                lhsT=klT_sb[:, j, w * 128:(w + 1) * 128],
                                rhs=kth[:, j, bh * 128:(bh + 1) * 128],
                                start=(j == 0), stop=(j == 3))
                        eb = ebpool.tile([128, 128], bf16, tag="eb")
                        nc.scalar.activation(eb[:], M_ps[:], AF.Exp,
                                             scale=1.0 / TEMP)
                        em = ebpool.tile([128, 128], bf16, tag="em")
                        nc.vector.tensor_tensor(
                            em[:], eb[:],
                            qtm[:, b * 128:(b + 1) * 128], op=OP.mult)
                        T_ps = pMT.tile([128, 128], bf16, tag="T")
                        nc.tensor.transpose(T_ps[:], em[:], ident_sb[:])
                        et = ohpool.tile([128, 128], bf16, tag="et")
                        if b % 2 == 0:
                            nc.scalar.copy(et[:], T_ps[:])
                        else:
                            nc.vector.tensor_copy(et[:], T_ps[:])
                        # eh per edge = row-sum of E^T (raw, pre-normalize)
                        nc.vector.tensor_reduce(
                            ehs[:, b:b + 1], et[:],
                            axis=mybir.AxisListType.X, op=OP.add)
                        ets.append(et)
                    # escore shard write: esc[w, p, b] = eh[p, b]
                    nc.sync.dma_start(esc_sh[w, :, :], ehs[:])

                    for cc in range(4):
                        for b in range(BPW):
                            nc.tensor.matmul(
                                msgT_ps[:, cc * 128:(cc + 1) * 128],
                                lhsT=vg[:, b, cc * 128:(cc + 1) * 128],
                                rhs=ets[b][:],
                                start=(b == 0), stop=(b == BPW - 1))
                    for b in range(BPW):
                        nc.tensor.matmul(
                            z_ps[0:1, :], lhsT=ones_col[:], rhs=ets[b][:],
                            start=(b == 0), stop=(b == BPW - 1))

                    _window_tail(nc, tc, mybir, w, msgT_ps, z_ps,
                                 tp, paux, ph, ones_row, woT_sb, bo_sb,
                                 hacc, W)
                for oc in ([] if (SKIP_C or SKIP_L) else range(4)):
                    nc.sync.dma_start(
                        hT_out["L"][oc * 128:(oc + 1) * 128, :],
                        hacc[:, oc * npad:(oc + 1) * npad])

                # prefetch the first R v-gathers (need only AG#2) so
                # their transfers overlap the L-loop tail and AG#3
                vpre = []
                for w in ([] if (SKIP_C or SKIP_R) else range(2)):
                    v = gpool.tile([128, BPW, C], bf16, tag="v")
                    nc.gpsimd.dma_gather(
                        v[:], tv_L[:, :], vidx_sb[:, w * 64:(w + 1) * 64],
                        WCAP, nidx_reg, C)
                    vpre.append(v)

                # ---- AG#3: escore ----
                if not SKIP_C:
                    if not SKIP_AG:
                        nc.gpsimd.collective_compute(
                            "AllGather", mybir.AluOpType.bypass,
                            replica_groups=[list(range(NCORES))],
                            ins=[esc_sh], outs=[esc_full])
                    else:
                        nc.sync.dma_start(esc_full[0:2, :],
                                          esc_sh[0, 0:16, :])

                # ---- phase C-R: right messages from shared scores ----
                for w in ([] if (SKIP_C or SKIP_R) else range(W)):
                    if w < len(vpre):
                        v = vpre[w]
                    else:
                        v = gpool.tile([128, BPW, C], bf16, tag="v")
                        nc.gpsimd.dma_gather(
                            v[:], tv_L[:, :], vidx_sb[:, w * 64:(w + 1) * 64],
                            WCAP, nidx_reg, C)
                    eg = kdpool.tile([128, BPW, 64], f32, tag="eg")
                    nc.gpsimd.dma_gather(
                        eg[:], esc_full[:, :], eidx_sb[:, w * 64:(w + 1) * 64],
                        WCAP, nidx_reg, 64, queue_num=1)

                    # select each edge's eh via the host-shipped one-hot mask
                    egm = sp.tile([128, BPW, 64], bf16, tag="egm")
                    nc.vector.tensor_tensor(
                        egm[:], eg[:, :, :],
                        emask_sb[:, w * BPW * 64:(w + 1) * BPW * 64],
                        op=OP.mult)
                    ehR = sp.tile([128, BPW], f32, tag="ehR")
                    nc.vector.tensor_reduce(
                        ehR[:], egm[:, :, :],
                        axis=mybir.AxisListType.X, op=OP.add)

                    ohs = []
                    for b in range(BPW):
                        oh = ohpool.tile([128, 128], bf16, tag="et")
                        nc.vector.tensor_scalar(
                            oh[:], iota_sb[:],
                            drelR_sb[:, w * BPW + b: w * BPW + b + 1],
                            ehR[:, b:b + 1], op0=OP.is_equal, op1=OP.mult)
                        ohs.append(oh)

                    msgT_ps = pmsg.tile([128, 512], f32)
                    z_ps = paux.tile([128, 128], f32, tag="aux")
                    for cc in range(4):
                        for b in range(BPW):
                            nc.tensor.matmul(
                                msgT_ps[:, cc * 128:(cc + 1) * 128],
                                lhsT=v[:, b, cc * 128:(cc + 1) * 128],
                                rhs=ohs[b][:],
                                start=(b == 0), stop=(b == BPW - 1))
                    for b in range(BPW):
                        nc.tensor.matmul(
                            z_ps[0:1, :], lhsT=ones_col[:], rhs=ohs[b][:],
                            start=(b == 0), stop=(b == BPW - 1))

                    _window_tail(nc, tc, mybir, w, msgT_ps, z_ps,
                                 tp, paux, ph, ones_row, woT_sb, bo_sb,
                                 hacc, W)
                for oc in ([] if (SKIP_C or SKIP_R) else range(4)):
                    nc.sync.dma_start(
                        hT_out["R"][oc * 128:(oc + 1) * 128, :],
                        hacc[:, oc * npad:(oc + 1) * npad])
    nc.compile()
    return nc


def _window_tail(nc, tc, mybir, w, msgT_ps, z_ps, tp, paux, ph,
                 ones_row, woT_sb, bo_sb, hacc, W):
    """z -> 1/z broadcast, msgT normalize, Wo GEMM, bias+LeakyReLU."""
    f32, bf16 = mybir.dt.float32, mybir.dt.bfloat16
    AF = mybir.ActivationFunctionType
    OP = mybir.AluOpType
    npad = W * 128

    zm = tp.tile([1, 128], f32, tag="zm")
    nc.vector.tensor_scalar_max(zm[:], z_ps[0:1, :], 1e-30)
    zr = tp.tile([1, 128], f32, tag="zr")
    nc.vector.reciprocal(zr[:], zm[:])
    zrb = tp.tile([1, 128], bf16, tag="zrb")
    nc.vector.tensor_copy(zrb[:], zr[:])
    zbc_ps = paux.tile([128, 128], f32, tag="aux")
    nc.tensor.matmul(zbc_ps[:], lhsT=ones_row[:], rhs=zrb[:],
                     start=True, stop=True)
    zbc = tp.tile([128, 128], f32, tag="zbc")
    nc.scalar.copy(zbc[:], zbc_ps[:])
    msgT_sb = tp.tile([128, 512], bf16, tag="msgT")
    for nch in range(4):
        nc.vector.tensor_tensor(
            msgT_sb[:, nch * 128:(nch + 1) * 128],
            msgT_ps[:, nch * 128:(nch + 1) * 128],
            zbc[:], op=OP.mult)
    hT_ps = ph.tile([128, 512], f32)
    for oc in range(4):
        for cc in range(4):
            nc.tensor.matmul(
                hT_ps[:, oc * 128:(oc + 1) * 128],
                lhsT=woT_sb[:, cc * 512 + oc * 128:
                            cc * 512 + oc * 128 + 128],
                rhs=msgT_sb[:, cc * 128:(cc + 1) * 128],
                start=(cc == 0), stop=(cc == 3))
    for oc in range(4):
        x = hacc[:, oc * npad + w * 128: oc * npad + (w + 1) * 128]
        nc.scalar.activation(
            x, hT_ps[:, oc * 128:(oc + 1) * 128],
            AF.Identity, bias=bo_sb[:, oc:oc + 1])
        x2 = tp.tile([128, 128], bf16, tag="x2")
        nc.vector.tensor_scalar_mul(x2[:], x, NEG)
        nc.vector.tensor_tensor(x, x, x2[:], op=OP.max)


def _assemble(results, arrs, key):
    out = np.zeros((N, C), np.float32)
    for c in range(NCORES):
        hT = np.asarray(results[c][key], np.float32)
        cn = arrs[c]["colnode"]
        m = cn >= 0
        out[cn[m]] = hT[:, m].T
    return out


_RUN_KWARGS = {}


def kernel(**inputs):
    from concourse.bass_utils import run_bass_kernel_spmd

    in_maps, arrL, arrR, W = _host_inputs(inputs)
    nc = _build_program(W)
    res = run_bass_kernel_spmd(nc, in_maps, core_ids=list(range(NCORES)),
                               **_RUN_KWARGS)
    out_l = _assemble(res.results, arrL, "hT_L")
    out_r = _assemble(res.results, arrR, "hT_R")
    kernel.last_results = res
    kernel.last_nc = nc
    kernel.last_W = W
    return (out_l, out_r)



# revision 10
# speedup vs baseline: 1.0387x; 1.0387x over previous
"""Trainium2 Bass kernel for nn_MultiHeadAttention_73589969649754
(gnn_message_passing / graph cross-attention).

v3 strategy (score sharing + matmul scoring):
  - Edges sorted by destination per side; core c owns nodes
    [c*2500, (c+1)*2500) on both sides.  Windows of <=128 consecutive
    nodes / <=1024 edge slots.  The HOST permutes each core's node
    columns so window w occupies slots [w*128, (w+1)*128) - all program
    offsets are SPMD-uniform and tables are slot-ordered.
  - Phase A: R side emits a fused K|V table (AllGather #1, 5.24MB/rank);
    L side computes K TRANSPOSED ([channel, slot], kept in SBUF - no
    DRAM round trip) and a V table (AllGather #2, 2.62MB/rank).
  - L pass per window: one TRANSPOSED dma_gather pulls Kr[src] in
    [channel, edge] orientation and a normal gather pulls Vr[src].
    Scores come from PE matmuls M[slot, edge] = KlT_win^T @ KrT_gath
    (no per-edge dot product on DVE/ACT), then exp on ACT, a host-shipped
    one-hot mask zeroes off-segment entries, and a PE transpose yields
    E^T[edge, slot] - which IS the eh-scaled one-hot the segment-sum
    matmuls consume.  eh per edge (row-sum of E^T) is written to a score
    shard (AllGather #3, 80KB/rank).  z via ones-matmul; messages are
    normalized after the fact by 1/z (column broadcast); Wo GEMM;
    bias+LeakyReLU.
  - R pass: gathers only Vl[src] rows (1KB/edge) plus 256B score-table
    rows; a host-shipped one-hot mask selects each edge's eh (softmax
    numerators are shared between the two sides).  No K gather, no dot
    product, no exp on the R side.
  - Outputs stored bf16 in [channel, slot] layout; host reassembles.
"""

import math

import numpy as np

N = 20000
E = 160000
C = 512
NCORES = 8
TEMP = float(np.sqrt(C))
NEG = 0.01
NPC = N // NCORES            # 2500 nodes per core per side
BLK = 128                    # edges per block
BPW = 8                      # blocks per window
WCAP = BPW * BLK             # 1024 edge slots per window
DUMMY_REL = 999.0
SKIP_AG = False
SKIP_C = False
SKIP_L = False       # debug: skip the L window loop
SKIP_R = False       # debug: skip the R window loop

# AllGather wall-time charge (ns): measured-table upper bound is ~70us for
# a 5.24MB/rank 8-way intra-chip AllGather; scale by bytes with a 20us
# latency floor.  Serial charges: AG#1 tkv_R 5.24MB -> 70us (gates the L
# loop) and AG#3 escore 80KB -> 20us (floor; gates the R score gathers).
# AG#2 (tv_L, 2.62MB -> ~35us) runs on the collective cores/links right
# after AG#1 (~105us done) and is only consumed by the R loop, which
# starts after the ~175us L loop - fully hidden, so not charged.
AG_CHARGE_NS = 70e3 + 20e3


def _prep_side(seg_dst):
    """Sort edges by dst; node-aligned core ranges; pack windows."""
    seg_dst = np.asarray(seg_dst, np.int64)
    perm = np.argsort(seg_dst, kind="stable")
    sd = seg_dst[perm]
    deg = np.bincount(sd, minlength=N)
    edge_b = [int(np.searchsorted(sd, c * NPC, "left"))
              for c in range(NCORES)] + [E]

    cores = []
    max_w = 0
    for c in range(NCORES):
        n0, n1 = c * NPC, (c + 1) * NPC
        e0 = edge_b[c]
        wins = []
        n, e = n0, e0
        while n < n1:
            wn = we = 0
            while n + wn < n1 and wn < BLK and we + deg[n + wn] <= WCAP:
                we += deg[n + wn]
                wn += 1
            assert wn > 0, "node degree exceeds window capacity"
            wins.append((n, wn, e, we))
            n += wn
            e += we
        assert e == edge_b[c + 1]
        cores.append((wins, e0))
        max_w = max(max_w, len(wins))
    return perm, sd, cores, max_w


def _wrap_idx16(idx_flat):
    """[n] -> [128, n//16] int16, i at [i%16, i//16], replicated x8."""
    n = idx_flat.shape[0]
    a = idx_flat.reshape(n // 16, 16).T.astype(np.int16)
    return np.ascontiguousarray(np.tile(a, (8, 1)))


def _slot_maps(cores, W):
    """Slot-order the nodes: window w of core c occupies slots
    [w*128, (w+1)*128).  Returns (slot_node [NCORES, W*128] node-or--1,
    node_slot [N] global slot = core*W*128 + slot)."""
    npad = W * BLK
    slot_node = np.full((NCORES, npad), -1, np.int64)
    node_slot = np.full(N, -1, np.int64)
    for c, (wins, e0) in enumerate(cores):
        for w, (fn, wn, es, ne) in enumerate(wins):
            sl = np.arange(wn)
            slot_node[c, w * BLK + sl] = fn + sl
            node_slot[fn + sl] = c * npad + w * BLK + sl
    assert (node_slot >= 0).all()
    return slot_node, node_slot


def _edge_locs(perm, cores):
    """Per original edge id: (core, window, slot-in-window) on this side."""
    core_of = np.empty(E, np.int32)
    win_of = np.empty(E, np.int32)
    slot_of = np.empty(E, np.int32)
    for c, (wins, e0) in enumerate(cores):
        for w, (fn, wn, es, ne) in enumerate(wins):
            ids = perm[es:es + ne]
            core_of[ids] = c
            win_of[ids] = w
            slot_of[ids] = np.arange(ne)
    return core_of, win_of, slot_of


def _host_inputs(inputs):
    import ml_dtypes
    bf16 = ml_dtypes.bfloat16

    nl = np.asarray(inputs["node_left"], np.float32)
    nr = np.asarray(inputs["node_right"], np.float32)
    Wk = np.asarray(inputs["Wk"], np.float32)
    Wv = np.asarray(inputs["Wv"], np.float32)
    Wo = np.asarray(inputs["Wo"], np.float32)
    bo = np.asarray(inputs["bo"], np.float32)
    sl = np.asarray(inputs["segmentation_index_left"], np.int64)
    sr = np.asarray(inputs["segmentation_index_right"], np.int64)

    permL, sdL, coresL, wL = _prep_side(sl)
    permR, sdR, coresR, wR = _prep_side(sr)
    W = max(wL, wR)
    npad = W * BLK

    snL, nsL = _slot_maps(coresL, W)     # left-node slots
    snR, nsR = _slot_maps(coresR, W)     # right-node slots
    LcoreE, LwinE, LslotE = _edge_locs(permL, coresL)

    # ---- per-core L arrays ----
    ssL = sr[permL]          # src (right) node per L-sorted edge
    arrL = []
    for c, (wins, e0) in enumerate(coresL):
        sidx = np.zeros((W, WCAP), np.int64)      # src rows in R table
        drel = np.full((W, WCAP), DUMMY_REL, np.float32)
        for w, (fn, wn, es, ne) in enumerate(wins):
            sidx[w, :ne] = nsR[ssL[es:es + ne]]
            rel = (sdL[es:es + ne] - fn)
            drel[w, :ne] = rel.astype(np.float32)
        sidx16 = np.concatenate([_wrap_idx16(sidx[w]) for w in range(W)], 1)
        drelT = np.ascontiguousarray(
            drel.reshape(W, BPW, BLK).transpose(2, 0, 1).reshape(BLK, W * BPW))
        arrL.append(dict(sidx=sidx16, drel=drelT, colnode=snL[c]))

    # ---- per-core R arrays ----
    ssR = sl[permR]          # src (left) node per R-sorted edge
    arrR = []
    for c, (wins, e0) in enumerate(coresR):
        vidx = np.zeros((W, WCAP), np.int64)      # src rows in L V table
        eidx = np.zeros((W, WCAP), np.int64)      # escore row (64-col rows)
        ecol = np.zeros((W, WCAP), np.int64)
        emask_valid = np.zeros((W, WCAP), bool)
        drel = np.full((W, WCAP), DUMMY_REL, np.float32)
        for w, (fn, wn, es, ne) in enumerate(wins):
            ids = permR[es:es + ne]
            vidx[w, :ne] = nsL[ssR[es:es + ne]]
            # flat L escore position: core*(W*1024) + win*1024 + p*8 + b
            pL = LslotE[ids] % BLK
            bL = LslotE[ids] // BLK
            gpos = (LcoreE[ids].astype(np.int64) * W + LwinE[ids]) * WCAP \
                + pL * BPW + bL
            eidx[w, :ne] = gpos // 64
            ecol[w, :ne] = gpos % 64
            emask_valid[w, :ne] = True
            drel[w, :ne] = (sdR[es:es + ne] - fn).astype(np.float32)
        vidx16 = np.concatenate([_wrap_idx16(vidx[w]) for w in range(W)], 1)
        eidx16 = np.concatenate([_wrap_idx16(eidx[w]) for w in range(W)], 1)
        drelT = np.ascontiguousarray(
            drel.reshape(W, BPW, BLK).transpose(2, 0, 1).reshape(BLK, W * BPW))
        emask = np.zeros((BLK, W * BPW * 64), np.float32)
        wv, iv = np.nonzero(emask_valid)
        pv, bv = iv % BLK, iv // BLK
        emask[pv, (wv * BPW + bv) * 64 + ecol[wv, iv]] = 1.0
        arrR.append(dict(vidx=vidx16, eidx=eidx16, drel=drelT,
                         emask=emask.astype(bf16), colnode=snR[c]))

    # ---- shared constants ----
    Wkv = np.concatenate([Wk, Wv], 0)               # [1024, 512]
    WkvT = Wkv.T                                    # [512, 1024]
    wkvT_arr = np.zeros((128, 4 * 1024), np.float32)
    for cc in range(4):
        wkvT_arr[:, cc * 1024:(cc + 1) * 1024] = \
            WkvT[cc * 128:(cc + 1) * 128, :]
    # wkT: lhsT tiles for the transposed K GEMM
    wkT_arr = np.zeros((128, 16 * 128), np.float32)
    for i in range(4):
        for o in range(4):
            wkT_arr[:, (i * 4 + o) * 128:(i * 4 + o + 1) * 128] = \
                Wk[o * 128:(o + 1) * 128, i * 128:(i + 1) * 128].T
    woT_arr = np.zeros((128, 4 * 512), np.float32)
    for cc in range(4):
        for oc in range(4):
            woT_arr[:, cc * 512 + oc * 128: cc * 512 + (oc + 1) * 128] = \
                Wo[oc * 128:(oc + 1) * 128, cc * 128:(cc + 1) * 128].T
    bo_arr = bo.reshape(1, 512).copy()              # [1, 512] oc-major
    iota_arr = np.broadcast_to(
        np.arange(128, dtype=np.float32)[None, :], (128, 128))

    def shardT(feat, slot_node_c):
        sh = np.zeros((C, npad), np.float32)
        m = slot_node_c >= 0
        sh[:, m] = feat[slot_node_c[m]].T
        return np.ascontiguousarray(sh).astype(bf16)

    in_maps = []
    for c in range(NCORES):
        in_maps.append({
            "nT_L": shardT(nl, snL[c]),
            "nT_R": shardT(nr, snR[c]),
            "wkvT": wkvT_arr.astype(bf16),
            "wkT": wkT_arr.astype(bf16),
            "woT": woT_arr.astype(bf16),
            "bo": bo_arr.astype(bf16),
            "iota": np.ascontiguousarray(iota_arr).astype(bf16),
            "sidx_L": arrL[c]["sidx"],
            "drel_L": arrL[c]["drel"],
            "vidx_R": arrR[c]["vidx"],
            "eidx_R": arrR[c]["eidx"],
            "drel_R": arrR[c]["drel"],
            "emask_R": arrR[c]["emask"],
        })
    return in_maps, arrL, arrR, W


def _build_program(W):
    import concourse.bacc as bacc
    import concourse.tile as tile
    from concourse import mybir

    dt = mybir.dt
    f32, bf16, i16 = dt.float32, dt.bfloat16, dt.int16
    AF = mybir.ActivationFunctionType
    OP = mybir.AluOpType
    npad = W * BLK
    ntiles = W

    nc = bacc.Bacc("TRN2", target_bir_lowering=False, debug=False,
                   enable_asserts=True, num_devices=NCORES,
                   num_swdge_queues=2)

    # ---- I/O ----
    nT_in = {s: nc.dram_tensor(f"nT_{s}", [C, npad], bf16,
                               kind="ExternalInput").ap() for s in "LR"}
    wkvT = nc.dram_tensor("wkvT", [128, 4 * 1024], bf16,
                          kind="ExternalInput").ap()
    wkT_in = nc.dram_tensor("wkT", [128, 16 * 128], bf16,
                            kind="ExternalInput").ap()
    woT = nc.dram_tensor("woT", [128, 4 * 512], bf16,
                         kind="ExternalInput").ap()
    bo_in = nc.dram_tensor("bo", [1, 512], bf16,
                          kind="ExternalInput").ap()
    iota_in = nc.dram_tensor("iota", [128, 128], bf16,
                             kind="ExternalInput").ap()
    sidx_in = nc.dram_tensor("sidx_L", [128, W * 64], i16,
                             kind="ExternalInput").ap()
    drelL_in = nc.dram_tensor("drel_L", [128, W * BPW], f32,
                              kind="ExternalInput").ap()
    vidx_in = nc.dram_tensor("vidx_R", [128, W * 64], i16,
                             kind="ExternalInput").ap()
    eidx_in = nc.dram_tensor("eidx_R", [128, W * 64], i16,
                             kind="ExternalInput").ap()
    drelR_in = nc.dram_tensor("drel_R", [128, W * BPW], f32,
                              kind="ExternalInput").ap()
    emask_in = nc.dram_tensor("emask_R", [128, W * BPW * 64], bf16,
                              kind="ExternalInput").ap()
    hT_out = {s: nc.dram_tensor(f"hT_{s}", [C, npad], bf16,
                                kind="ExternalOutput").ap() for s in "LR"}

    # ---- internal DRAM ----
    tkv_shR = nc.dram_tensor("tkv_shR", [npad, 2 * C], bf16).ap()
    tv_shL = nc.dram_tensor("tv_shL", [npad, C], bf16).ap()
    esc_sh = nc.dram_tensor("esc_sh", [W, 128, BPW], f32).ap()
    tkv_R = nc.dram_tensor("tkv_R", [NCORES * npad, 2 * C], bf16,
                           addr_space="Shared").ap()
    tv_L = nc.dram_tensor("tv_L", [NCORES * npad, C], bf16,
                          addr_space="Shared").ap()
    esc_full = nc.dram_tensor("esc_full", [NCORES * W * 16, 64], f32,
                              addr_space="Shared").ap()

    with tile.TileContext(nc) as tc:
        with tc.tile_pool(name="const", bufs=1) as cpool:
            # early constants (phase A + L loop)
            wkvT_sb = cpool.tile([128, 4 * 1024], bf16)
            nc.sync.dma_start(wkvT_sb[:], wkvT[:, :])
            sidx_sb = cpool.tile([128, W * 64], i16)
            nc.sync.dma_start(sidx_sb[:], sidx_in[:, :])
            drelL_sb = cpool.tile([128, W * BPW], f32)
            nc.sync.dma_start(drelL_sb[:], drelL_in[:, :])
            woT_sb = cpool.tile([128, 4 * 512], bf16)
            nc.sync.dma_start(woT_sb[:], woT[:, :])
            bo_sb = cpool.tile([1, 512], bf16)
            nc.sync.dma_start(bo_sb[:], bo_in[:, :])
            iota_sb = cpool.tile([128, 128], bf16)
            nc.sync.dma_start(iota_sb[:], iota_in[:, :])
            ones_col = cpool.tile([128, 1], bf16)
            nc.vector.memset(ones_col[:], 1.0)
            ones_row = cpool.tile([1, 128], bf16)
            nc.vector.memset(ones_row[:], 1.0)
            # late constants (loaded after AG#1/AG#2 are in flight)
            wkT_sb = cpool.tile([128, 16 * 128], bf16)
            vidx_sb = cpool.tile([128, W * 64], i16)
            eidx_sb = cpool.tile([128, W * 64], i16)
            drelR_sb = cpool.tile([128, W * BPW], f32)
            hacc = cpool.tile([128, 4, npad], bf16)
            klT_sb = cpool.tile([128, 4, npad], bf16)   # left K transposed

            # ---- phase A ----
            with (
                tc.tile_pool(name="feat", bufs=1) as fpool,
                tc.tile_pool(name="gemm_sb", bufs=3) as gsb,
                tc.tile_pool(name="psum_gemm", bufs=2, space="PSUM") as pg,
            ):
                featR = []
                for cc in range(4):
                    t = fpool.tile([128, npad], bf16, tag=f"featR{cc}")
                    nc.sync.dma_start(
                        t[:], nT_in["R"][cc * 128:(cc + 1) * 128, :])
                    featR.append(t)
                # R side K|V fused GEMM -> tkv_shR -> AG#1
                for ti in range(ntiles):
                    sb = gsb.tile([128, 1024], bf16)
                    for half in range(2):
                        ps = pg.tile([128, 512], f32)
                        for cc in range(4):
                            nc.tensor.matmul(
                                ps[:],
                                lhsT=featR[cc][:, ti * 128:(ti + 1) * 128],
                                rhs=wkvT_sb[:, cc * 1024 + half * 512:
                                            cc * 1024 + half * 512 + 512],
                                start=(cc == 0), stop=(cc == 3))
                        if half == 0:
                            nc.vector.tensor_copy(
                                sb[:, 0:512], ps[:])
                        else:
                            nc.scalar.copy(
                                sb[:, 512:1024], ps[:])
                    nc.sync.dma_start(
                        tkv_shR[ti * 128:(ti + 1) * 128, :], sb[:])
                if not SKIP_AG:
                    nc.gpsimd.collective_compute(
                        "AllGather", mybir.AluOpType.bypass,
                        replica_groups=[list(range(NCORES))],
                        ins=[tkv_shR], outs=[tkv_R])
                else:
                    # timing build: sliver copy keeps the dependency edge
                    nc.sync.dma_start(tkv_R[0:128, :], tkv_shR[0:128, :])

                featL = []
                for cc in range(4):
                    t = fpool.tile([128, npad], bf16, tag=f"featL{cc}")
                    nc.sync.dma_start(
                        t[:], nT_in["L"][cc * 128:(cc + 1) * 128, :])
                    featL.append(t)
                # L side V GEMM -> tv_shL -> AG#2
                for ti in range(ntiles):
                    sb = gsb.tile([128, 512], bf16, tag="sbv")
                    ps = pg.tile([128, 512], f32)
                    for cc in range(4):
                        nc.tensor.matmul(
                            ps[:],
                            lhsT=featL[cc][:, ti * 128:(ti + 1) * 128],
                            rhs=wkvT_sb[:, cc * 1024 + 512:
                                        cc * 1024 + 1024],
                            start=(cc == 0), stop=(cc == 3))
                    nc.vector.tensor_copy(sb[:], ps[:])
                    nc.sync.dma_start(
                        tv_shL[ti * 128:(ti + 1) * 128, :], sb[:])
                if not SKIP_AG:
                    nc.gpsimd.collective_compute(
                        "AllGather", mybir.AluOpType.bypass,
                        replica_groups=[list(range(NCORES))],
                        ins=[tv_shL], outs=[tv_L])
                else:
                    nc.sync.dma_start(tv_L[0:128, :], tv_shL[0:128, :])

                # L transposed-K GEMM, AFTER both AGs are in flight: on HW
                # this PE work (plus the late const loads below) overlaps
                # the AG#1 link time, shrinking its serial charge.
                nc.sync.dma_start(wkT_sb[:], wkT_in[:, :])
                for o in range(4):
                    for nb in range(math.ceil(npad / 512)):
                        n0 = nb * 512
                        n1 = min(npad, n0 + 512)
                        ps = pg.tile([128, 512], f32)
                        for i in range(4):
                            nc.tensor.matmul(
                                ps[:, 0:n1 - n0],
                                lhsT=wkT_sb[:, (i * 4 + o) * 128:
                                            (i * 4 + o + 1) * 128],
                                rhs=featL[i][:, n0:n1],
                                start=(i == 0), stop=(i == 3))
                        nc.scalar.copy(
                            klT_sb[:, o, n0:n1], ps[:, 0:n1 - n0])

            # late const loads (R loop)
            nc.sync.dma_start(vidx_sb[:], vidx_in[:, :])
            nc.sync.dma_start(eidx_sb[:], eidx_in[:, :])
            nc.sync.dma_start(drelR_sb[:], drelR_in[:, :])

            nidx_reg = nc.gpsimd.to_reg(WCAP)
            nidx_reg2 = nc.gpsimd.to_reg(WCAP // 2)

            def window_tail(w, msgT_ps, zt, tp, ph):
                """1/z -> broadcast, normalize, Wo GEMM + bias, LeakyReLU.
                zt is a [128, 512] PSUM tile whose row 0 cols 0:128 hold z;
                it is then overwritten with the broadcast 1/z."""
                zm = tp.tile([1, 128], f32, tag="zm")
                nc.vector.tensor_scalar_max(zm[:], zt[0:1, 0:128], 1e-30)
                zr = tp.tile([1, 128], f32, tag="zr")
                nc.vector.reciprocal(zr[:], zm[:])
                zrb = tp.tile([1, 128], bf16, tag="zrb")
                nc.vector.tensor_copy(zrb[:], zr[:])
                for cc in range(4):
                    nc.tensor.matmul(
                        zt[:, cc * 128:(cc + 1) * 128],
                        lhsT=ones_row[:], rhs=zrb[:],
                        start=True, stop=True)
                zbc = tp.tile([128, 512], bf16, tag="zbcs")
                nc.scalar.copy(zbc[:], zt[:])
                msgn = tp.tile([128, 512], bf16, tag="msgn")
                nc.vector.tensor_tensor(
                    msgn[:], msgT_ps[:], zbc[:], op=OP.mult)
                hT_ps = ph.tile([128, 4, 128], f32, tag="hT")
                for oc in range(4):
                    for cc in range(4):
                        nc.tensor.matmul(
                            hT_ps[:, oc, :],
                            lhsT=woT_sb[:, cc * 512 + oc * 128:
                                        cc * 512 + oc * 128 + 128],
                            rhs=msgn[:, cc * 128:(cc + 1) * 128],
                            start=(cc == 0), stop=False)
                    # bias as a rank-1 accumulate closing the PSUM group
                    nc.tensor.matmul(
                        hT_ps[:, oc, :],
                        lhsT=bo_sb[:, oc * 128:(oc + 1) * 128],
                        rhs=ones_row[:],
                        start=False, stop=True)
                # bias already added; one fused LeakyReLU over all 4 chunks
                nc.scalar.activation(
                    hacc[:, :, w * 128:(w + 1) * 128], hT_ps[:],
                    AF.Lrelu, alpha=NEG)

            with (
                tc.tile_pool(name="gath", bufs=3) as gpool,
                tc.tile_pool(name="egp", bufs=3) as kdpool,
                tc.tile_pool(name="vpre", bufs=7) as vppool,
                tc.tile_pool(name="blk", bufs=4) as sp,
                tc.tile_pool(name="ebuf", bufs=3) as ebpool,
                tc.tile_pool(name="etbuf", bufs=3) as ohpool,
                tc.tile_pool(name="tail", bufs=3) as tp,
                tc.tile_pool(name="pMT", bufs=2, space="PSUM") as pMT,
                tc.tile_pool(name="pmsg", bufs=2, space="PSUM") as pmsg,
                tc.tile_pool(name="ph", bufs=1, space="PSUM") as ph,
            ):
                # ---- phase C-L: transposed matmul scores + left messages ----
                for w in ([] if (SKIP_C or SKIP_L) else range(W)):
                    # transposed K gather split in two: the worker's ucode
                    # fails above 512 indices per transpose gather
                    ktgs = []
                    for h in range(2):
                        kt = gpool.tile([128, 4, WCAP // 2], bf16,
                                        tag=f"ktg{h}", bufs=2)
                        nc.gpsimd.dma_gather(
                            kt[:], tkv_R[:, 0:C],
                            sidx_sb[:, w * 64 + h * 32: w * 64 + h * 32 + 32],
                            WCAP // 2, nidx_reg2, C, elem_step=2 * C,
                            transpose=True)
                        ktgs.append(kt)
                    vg = gpool.tile([128, BPW, C], bf16, tag="vg", bufs=2)
                    nc.gpsimd.dma_gather(
                        vg[:], tkv_R[:, C:2 * C],
                        sidx_sb[:, w * 64:(w + 1) * 64],
                        WCAP, nidx_reg, C, elem_step=2 * C, queue_num=1)

                    # scores TRANSPOSED: MT[e, slot] = Kr_g^T @ Kl_win
                    MTs = []
                    for h in range(2):
                        MT = pMT.tile([128, 4, 128], f32, tag=f"MT{h}")
                        for bh in range(4):
                            for j in range(4):
                                nc.tensor.matmul(
                                    MT[:, bh, :],
                                    lhsT=ktgs[h][:, j,
                                                 bh * 128:(bh + 1) * 128],
                                    rhs=klT_sb[:, j, w * 128:(w + 1) * 128],
                                    start=(j == 0), stop=(j == 3))
                        MTs.append(MT)
                    etmps = []
                    for h in range(2):
                        etmp = ebpool.tile([128, 4, 128], bf16,
                                           tag=f"etmp{h}")
                        nc.scalar.activation(etmp[:], MTs[h][:], AF.Exp,
                                             scale=1.0 / TEMP)
                        etmps.append(etmp)
                    # mask off-segment entries + row-reduce eh in one op
                    et = ohpool.tile([128, BPW, 128], bf16, tag="et")
                    ehs = sp.tile([128, BPW], f32, tag="ehs")
                    for b in range(BPW):
                        nc.vector.scalar_tensor_tensor(
                            et[:, b, :], iota_sb[:],
                            drelL_sb[:, w * BPW + b: w * BPW + b + 1],
                            etmps[b // 4][:, b % 4, :],
                            op0=OP.is_equal, op1=OP.mult,
                            accum_out=ehs[:, b:b + 1])
                    # escore shard write: esc[w, p, b] = eh[p, b]
                    nc.sync.dma_start(esc_sh[w, :, :], ehs[:])

                    msgT_ps = pmsg.tile([128, 512], f32)
                    zt = ph.tile([128, 512], f32, tag="zbc")
                    for cc in range(4):
                        for b in range(BPW):
                            nc.tensor.matmul(
                                msgT_ps[:, cc * 128:(cc + 1) * 128],
                                lhsT=vg[:, b, cc * 128:(cc + 1) * 128],
                                rhs=et[:, b, :],
                                start=(b == 0), stop=(b == BPW - 1))
                    for b in range(BPW):
                        nc.tensor.matmul(
                            zt[0:1, 0:128], lhsT=ones_col[:], rhs=et[:, b, :],
                            start=(b == 0), stop=(b == BPW - 1))

                    window_tail(w, msgT_ps, zt, tp, ph)
                for oc in ([] if (SKIP_C or SKIP_L) else range(4)):
                    nc.sync.dma_start(
                        hT_out["L"][oc * 128:(oc + 1) * 128, :],
                        hacc[:, oc, :])

                # prefetch R v-gathers (need only AG#2): their DMA time
                # covers AG#3's link+latency window
                vpre = []
                for w in ([] if (SKIP_C or SKIP_R) else range(7)):
                    v = vppool.tile([128, BPW, C], bf16, tag="v")
                    nc.gpsimd.dma_gather(
                        v[:], tv_L[:, :], vidx_sb[:, w * 64:(w + 1) * 64],
                        WCAP, nidx_reg, C)
                    vpre.append(v)

                # ---- AG#3: escore ----
                if not SKIP_C:
                    if not SKIP_AG:
                        nc.gpsimd.collective_compute(
                            "AllGather", mybir.AluOpType.bypass,
                            replica_groups=[list(range(NCORES))],
                            ins=[esc_sh], outs=[esc_full])
                    else:
                        nc.sync.dma_start(esc_full[0:2, :],
                                          esc_sh[0, 0:16, :])

                # ---- phase C-R: right messages from shared scores ----
                for w in ([] if (SKIP_C or SKIP_R) else range(W)):
                    if w < len(vpre):
                        v = vpre[w]
                    else:
                        v = vppool.tile([128, BPW, C], bf16, tag="v")
                        nc.gpsimd.dma_gather(
                            v[:], tv_L[:, :], vidx_sb[:, w * 64:(w + 1) * 64],
                            WCAP, nidx_reg, C)
                    eg = kdpool.tile([128, BPW, 64], f32, tag="eg")
                    nc.gpsimd.dma_gather(
                        eg[:], esc_full[:, :], eidx_sb[:, w * 64:(w + 1) * 64],
                        WCAP, nidx_reg, 64, queue_num=1)
                    emask = kdpool.tile([128, BPW, 64], bf16, tag="emask")
                    nc.sync.dma_start(
                        emask[:],
                        emask_in[:, w * BPW * 64:(w + 1) * BPW * 64])

                    # select each edge's eh via the host-shipped one-hot
                    # mask; fused multiply + row-reduce per block
                    ehR = sp.tile([128, BPW], f32, tag="ehR")
                    scr = sp.tile([128, 64], bf16, tag="scr")
                    for b in range(BPW):
                        nc.vector.scalar_tensor_tensor(
                            scr[:], eg[:, b, :], 1.0,
                            emask[:, b, :],
                            op0=OP.mult, op1=OP.mult,
                            accum_out=ehR[:, b:b + 1])

                    oht = ohpool.tile([128, BPW, 128], bf16, tag="et")
                    for b in range(BPW):
                        nc.vector.tensor_scalar(
                            oht[:, b, :], iota_sb[:],
                            drelR_sb[:, w * BPW + b: w * BPW + b + 1],
                            ehR[:, b:b + 1], op0=OP.is_equal, op1=OP.mult)

                    msgT_ps = pmsg.tile([128, 512], f32)
                    zt = ph.tile([128, 512], f32, tag="zbc")
                    for cc in range(4):
                        for b in range(BPW):
                            nc.tensor.matmul(
                                msgT_ps[:, cc * 128:(cc + 1) * 128],
                                lhsT=v[:, b, cc * 128:(cc + 1) * 128],
                                rhs=oht[:, b, :],
                                start=(b == 0), stop=(b == BPW - 1))
                    for b in range(BPW):
                        nc.tensor.matmul(
                            zt[0:1, 0:128], lhsT=ones_col[:], rhs=oht[:, b, :],
                            start=(b == 0), stop=(b == BPW - 1))

                    window_tail(w, msgT_ps, zt, tp, ph)
                for oc in ([] if (SKIP_C or SKIP_R) else range(4)):
                    nc.sync.dma_start(
                        hT_out["R"][oc * 128:(oc + 1) * 128, :],
                        hacc[:, oc, :])
    nc.compile()
    return nc


def _assemble(results, arrs, key):
    out = np.zeros((N, C), np.float32)
    for c in range(NCORES):
        hT = np.asarray(results[c][key], np.float32)
        cn = arrs[c]["colnode"]
        m = cn >= 0
        out[cn[m]] = hT[:, m].T
    return out


_RUN_KWARGS = {}


def kernel(**inputs):
    from concourse.bass_utils import run_bass_kernel_spmd

    in_maps, arrL, arrR, W = _host_inputs(inputs)
    nc = _build_program(W)
    res = run_bass_kernel_spmd(nc, in_maps, core_ids=list(range(NCORES)),
                               **_RUN_KWARGS)
    out_l = _assemble(res.results, arrL, "hT_L")
    out_r = _assemble(res.results, arrR, "hT_R")
    kernel.last_results = res
    kernel.last_nc = nc
    kernel.last_W = W
    return (out_l, out_r)

